# revision 1
# baseline (speedup 1.0000x reference)
"""MoE dense all-experts (GPT-OSS Experts forward) on 8 Trainium2 NeuronCores.

Expert-parallel sharding: core e holds expert e's weights and computes its
weighted contribution

    partial_e[t, h] = w[t, e] * ((up + 1) * silu(1.702 * gate) @ down_e.T + db_e)

with [gate | up] = hs @ gup_e + bias (the host de-interleaves gup's even/odd
columns so gate/up become contiguous halves). The token dimension is processed
in chunks; each chunk's partials are summed across the 8 cores with a
ReduceScatter that overlaps the next chunk's compute, and the host reassembles
the T-sharded RS outputs into the full [T, H] result.

Matmuls run in float32r (fp32 data on the PE at ~1 cycle/row, TF32-like
precision; end-to-end relative error ~2e-4). Stage 1 computes [f, t] tiles
(gate pass feeding the ScalarE Silu LUT, then up pass fused with the silu
output via scalar_tensor_tensor into act[i, t]); stage 2 computes out[t, h]
with act as the stationary operand. The down-bias + routing-weight epilogue
runs on the VectorE: out = (psum * w[t]) + w[t]*db[h], with the rank-1 w*db
tile built from a partition-broadcast copy of db.
"""
import sys
if '/opt/trn_rl_repo' not in sys.path:
    sys.path.insert(0, '/opt/trn_rl_repo')
import numpy as np

E, H, I, T = 8, 1024, 1024, 4096
N_CORES = 8
CHUNKS = [512] * 6 + [384, 384, 256]
KC = H // 128          # contraction chunks (H == I == 1024)
NJ = I // 128          # gate/up row tiles
TCMAX = max(CHUNKS)

_CACHE = {}


def _build():
    import concourse.bacc as bacc
    import concourse.tile as tile
    import concourse.mybir as mybir
    f32 = mybir.dt.float32
    f32r = mybir.dt.float32r
    AF = mybir.ActivationFunctionType
    ALU = mybir.AluOpType

    nc = bacc.Bacc("TRN2", target_bir_lowering=False, debug=False,
                   enable_asserts=False, num_devices=N_CORES)
    hsT = nc.dram_tensor("hsT", [H, T], f32r, kind="ExternalInput").ap()
    gup = nc.dram_tensor("gup", [H, 2 * I], f32r, kind="ExternalInput").ap()
    gb = nc.dram_tensor("gb", [128, NJ], f32, kind="ExternalInput").ap()
    ub = nc.dram_tensor("ub", [128, NJ], f32, kind="ExternalInput").ap()
    dwT = nc.dram_tensor("dwT", [I, H], f32r, kind="ExternalInput").ap()
    db = nc.dram_tensor("db", [1, H], f32, kind="ExternalInput").ap()
    wt = nc.dram_tensor("wt", [128, T // 128], f32, kind="ExternalInput").ap()
    osh = nc.dram_tensor("osh", [T // N_CORES, H], f32, kind="ExternalOutput").ap()
    otail = nc.dram_tensor("otail", [CHUNKS[-1], H], f32, kind="ExternalOutput").ap()

    with tile.TileContext(nc) as tc_:
        with tc_.tile_pool(name="wpool", bufs=1) as wpool, \
             tc_.tile_pool(name="hpool", bufs=2) as hpool, \
             tc_.tile_pool(name="apool", bufs=2) as apool, \
             tc_.tile_pool(name="spool", bufs=8) as spool, \
             tc_.tile_pool(name="opool", bufs=3) as opool, \
             tc_.tile_pool(name="bpool", bufs=4) as bpool, \
             tc_.tile_pool(name="dpool", bufs=2, space="DRAM") as dpool, \
             tc_.tile_pool(name="ps1", bufs=2, space="PSUM") as ps1, \
             tc_.tile_pool(name="ps2", bufs=3, space="PSUM") as ps2:

            gup_r = wpool.tile([128, KC * 2 * I], f32r)
            dwT_r = wpool.tile([128, KC * H], f32r)
            gb_r = wpool.tile([128, NJ], f32)
            ub_r = wpool.tile([128, NJ], f32)
            db_f = wpool.tile([1, H], f32)
            db_bc = wpool.tile([128, H], f32)
            w_r = wpool.tile([128, T // 128], f32)

            # DMA order matches consumption order: tiny bias/route tensors,
            # then per-kc (hs0, gate) pairs so the k-accumulation is DMA-paced,
            # then the up half, the chunk-1 token prefetch, and the down weights.
            nc.sync.dma_start(gb_r[:], gb[:])
            nc.sync.dma_start(ub_r[:], ub[:])
            nc.sync.dma_start(db_f[:], db[:])
            nc.sync.dma_start(w_r[:], wt[:])
            hs0 = hpool.tile([128, KC * TCMAX], f32r, tag="hs")
            for kc in range(KC):
                nc.sync.dma_start(hs0[:, kc*TCMAX:kc*TCMAX + CHUNKS[0]],
                                  hsT[kc*128:(kc+1)*128, 0:CHUNKS[0]])
                nc.sync.dma_start(gup_r[:, kc*2*I : kc*2*I + I],
                                  gup[kc*128:(kc+1)*128, 0:I])
            for kc in range(KC):
                nc.sync.dma_start(gup_r[:, kc*2*I + I : (kc+1)*2*I],
                                  gup[kc*128:(kc+1)*128, I:2*I])
            hs1 = hpool.tile([128, KC * TCMAX], f32r, tag="hs")
            nc.sync.dma_start(
                hs1[:].rearrange("p (kc t) -> p kc t", t=TCMAX)[:, :, 0:CHUNKS[1]],
                hsT[:, CHUNKS[0]:CHUNKS[0] + CHUNKS[1]].rearrange("(kc p) t -> p kc t", p=128))
            for kc in range(KC):
                nc.sync.dma_start(dwT_r[:, kc*H:(kc+1)*H], dwT[kc*128:(kc+1)*128, :])
            nc.gpsimd.partition_broadcast(db_bc[:], db_f[:])

            t_off = 0
            o_off = 0
            for c, TC in enumerate(CHUNKS):
                NTT = TC // 128
                OC = TC // N_CORES
                if c == 0:
                    hs_r = hs0
                elif c == 1:
                    hs_r = hs1
                else:
                    hs_r = hpool.tile([128, KC * TCMAX], f32r, tag="hs")
                    nc.sync.dma_start(
                        hs_r[:].rearrange("p (kc t) -> p kc t", t=TCMAX)[:, :, 0:TC],
                        hsT[:, t_off:t_off + TC].rearrange("(kc p) t -> p kc t", p=128))

                act_r = apool.tile([128, NJ * TCMAX], f32r, tag="act")
                s2s = []
                for j in range(NJ):     # gate pass
                    pg = ps1.tile([128, TC], f32, tag="pg")
                    for kc in range(KC):
                        nc.tensor.matmul(pg[:], gup_r[:, kc*2*I + j*128 : kc*2*I + (j+1)*128],
                                         hs_r[:, kc*TCMAX:kc*TCMAX + TC],
                                         start=(kc == 0), stop=(kc == KC - 1))
                    s2 = spool.tile([128, TCMAX], f32, tag="s2")
                    nc.scalar.activation(s2[:, :TC], pg[:], AF.Silu,
                                         bias=gb_r[:, j:j+1], scale=1.702)
                    s2s.append(s2)
                for j in range(NJ):     # up pass: act = (up + ub + 1) * silu_out
                    pu = ps1.tile([128, TC], f32, tag="pu")
                    for kc in range(KC):
                        nc.tensor.matmul(pu[:], gup_r[:, kc*2*I + I + j*128 : kc*2*I + I + (j+1)*128],
                                         hs_r[:, kc*TCMAX:kc*TCMAX + TC],
                                         start=(kc == 0), stop=(kc == KC - 1))
                    nc.vector.scalar_tensor_tensor(act_r[:, j*TCMAX:j*TCMAX + TC], pu[:],
                                                   ub_r[:, j:j+1], s2s[j][:, :TC],
                                                   op0=ALU.add, op1=ALU.mult)

                last = (c == len(CHUNKS) - 1)
                if not last:
                    bin_ = dpool.tile([TCMAX, H], f32, tag="bi")
                    bout = dpool.tile([TCMAX // N_CORES, H], f32, tag="bo")
                for tt in range(NTT):
                    gt = (t_off // 128) + tt
                    wcol = w_r[:, gt:gt+1]
                    ot = opool.tile([128, H], f32, tag="ot")
                    for hh in range(H // 512):
                        dbw = bpool.tile([128, 512], f32, tag="dbw")
                        nc.vector.tensor_scalar_mul(dbw[:], db_bc[:, hh*512:(hh+1)*512], wcol)
                        p2 = ps2.tile([128, 512], f32, tag="p2")
                        for ic in range(KC):
                            nc.tensor.matmul(p2[:], act_r[:, ic*TCMAX + tt*128 : ic*TCMAX + (tt+1)*128],
                                             dwT_r[:, ic*H + hh*512 : ic*H + (hh+1)*512],
                                             start=(ic == 0), stop=(ic == KC - 1))
                        nc.vector.scalar_tensor_tensor(ot[:, hh*512:(hh+1)*512], p2[:], wcol,
                                                       dbw[:], op0=ALU.mult, op1=ALU.add)
                    if last:
                        # final chunk: ship per-core partials; the host sums
                        # them in fp32 so the device tail ends at the prior RS
                        nc.sync.dma_start(otail[tt*128:(tt+1)*128, :], ot[:])
                    else:
                        nc.sync.dma_start(bin_[tt*128:(tt+1)*128, :], ot[:])
                if not last:
                    nc.gpsimd.collective_compute(
                        "ReduceScatter", ALU.add,
                        replica_groups=[list(range(N_CORES))],
                        ins=[bin_[:TC, :].opt()], outs=[bout[:OC, :].opt()])
                    nc.sync.dma_start(osh[o_off:o_off + OC, :], bout[:OC, :])
                t_off += TC
                o_off += OC
    nc.compile()
    return nc


def _get_nc():
    if 'nc' not in _CACHE:
        _CACHE['nc'] = _build()
    return _CACHE['nc']


def _make_in_maps(hidden_states, routing_weights, gate_up_proj, gate_up_proj_bias,
                  down_proj, down_proj_bias):
    hs = np.ascontiguousarray(np.asarray(hidden_states, dtype=np.float32))
    rw = np.asarray(routing_weights, dtype=np.float32)
    gupw = np.asarray(gate_up_proj, dtype=np.float32)
    gupb = np.asarray(gate_up_proj_bias, dtype=np.float32)
    dw = np.asarray(down_proj, dtype=np.float32)
    dbias = np.asarray(down_proj_bias, dtype=np.float32)
    hsT = np.ascontiguousarray(hs.T)
    in_maps = []
    for e in range(N_CORES):
        g = gupw[e]
        gup_de = np.concatenate([g[:, 0::2], g[:, 1::2]], axis=1)
        in_maps.append({
            "hsT": hsT,
            "gup": np.ascontiguousarray(gup_de),
            # silu(1.702*(x + b)) = silu(1.702*x + 1.702*b); the 1/1.702 glu
            # scale is folded into dwT below.
            "gb": np.ascontiguousarray((1.702 * gupb[e, 0::2]).reshape(NJ, 128).T),
            "ub": np.ascontiguousarray((gupb[e, 1::2] + 1.0).reshape(NJ, 128).T),
            "dwT": np.ascontiguousarray(dw[e].T / np.float32(1.702)),
            "db": np.ascontiguousarray(dbias[e][None, :]),
            "wt": np.ascontiguousarray(rw[:, e].reshape(T // 128, 128).T),
        })
    return in_maps


def _assemble(results):
    out = np.empty((T, H), dtype=np.float32)
    t_off = 0
    o_off = 0
    for TC in CHUNKS[:-1]:
        OC = TC // N_CORES
        for r in range(N_CORES):
            out[t_off + r*OC : t_off + (r+1)*OC, :] = results[r]["osh"][o_off:o_off + OC, :]
        t_off += TC
        o_off += OC
    out[t_off:, :] = np.sum([results[r]["otail"] for r in range(N_CORES)], axis=0)
    return out


def kernel(hidden_states, routing_weights, gate_up_proj, gate_up_proj_bias,
           down_proj, down_proj_bias):
    from concourse import bass_utils
    in_maps = _make_in_maps(hidden_states, routing_weights, gate_up_proj,
                            gate_up_proj_bias, down_proj, down_proj_bias)
    nc = _get_nc()
    try:
        res = bass_utils.run_bass_kernel_spmd(nc, in_maps, core_ids=list(range(N_CORES)))
    except Exception:
        # One retry in case a previous process left a core wedged.
        res = bass_utils.run_bass_kernel_spmd(nc, in_maps, core_ids=list(range(N_CORES)))
    return _assemble(res.results)



# revision 2
# speedup vs baseline: 1.3607x; 1.3607x over previous
"""MoE dense all-experts (GPT-OSS Experts forward) on 8 Trainium2 NeuronCores.

Expert-parallel sharding: core e holds expert e's weights and computes its
weighted contribution

    partial_e[t, h] = w[t, e] * ((up + 1) * silu(1.702 * gate) @ down_e.T)

with [gate | up] = hs @ gup_e + bias (the host de-interleaves gup's even/odd
columns so gate/up become contiguous halves). Each core writes its full
[T, H] partial to DRAM; the host sums the 8 partials and adds the
routing-weighted down-bias term (routing_weights @ down_bias) in fp32.

All matmul operands are float16 (fp32 PSUM accumulation): fp32/fp32r moving
operands stream through the PE at ~0.56 ns/col while 16-bit operands stream
at 1 col/cycle @ 2.4 GHz (0.417 ns/col), a 1.35x speedup at identical MAC
count; fp16's 10-bit mantissa keeps the end-to-end relative error ~5e-4.
Weights are staged in (j, kc)-block order so the gate pass starts after
~1.3 MB of DMA instead of waiting for the full weight tensor.

Stage 1 computes [f, t] tiles (gate pass feeding the ScalarE Silu LUT, then
up pass fused with the silu output via scalar_tensor_tensor into fp16
act[i, t]); stage 2 computes out[t, h] = (act.T @ dwT) * w[t] with act as
the stationary operand and a single VectorE tensor_scalar epilogue.
"""
import sys
if '/opt/trn_rl_repo' not in sys.path:
    sys.path.insert(0, '/opt/trn_rl_repo')
import numpy as np

E, H, I, T = 8, 1024, 1024, 4096
N_CORES = 8
TC = 512               # token chunk
NCHUNK = T // TC
KC = H // 128          # contraction chunks (H == I == 1024)
NJ = I // 128          # gate/up row tiles

_CACHE = {}


def _build():
    import concourse.bacc as bacc
    import concourse.tile as tile
    import concourse.mybir as mybir
    f32 = mybir.dt.float32
    f16 = mybir.dt.float16
    AF = mybir.ActivationFunctionType
    ALU = mybir.AluOpType

    nc = bacc.Bacc("TRN2", target_bir_lowering=False, debug=False,
                   enable_asserts=False, num_devices=N_CORES)
    hsT = nc.dram_tensor("hsT", [H, T], f16, kind="ExternalInput").ap()
    gupg = nc.dram_tensor("gupg", [128, NJ * KC * 128], f16, kind="ExternalInput").ap()
    gupu = nc.dram_tensor("gupu", [128, NJ * KC * 128], f16, kind="ExternalInput").ap()
    gb = nc.dram_tensor("gb", [128, NJ], f32, kind="ExternalInput").ap()
    ub = nc.dram_tensor("ub", [128, NJ], f32, kind="ExternalInput").ap()
    dwT = nc.dram_tensor("dwT", [128, KC * H], f16, kind="ExternalInput").ap()
    wt = nc.dram_tensor("wt", [128, T // 128], f32, kind="ExternalInput").ap()
    out = nc.dram_tensor("out", [T, H], f32, kind="ExternalOutput").ap()

    with tile.TileContext(nc) as tc_:
        with tc_.tile_pool(name="wpool", bufs=1) as wpool, \
             tc_.tile_pool(name="hpool", bufs=2) as hpool, \
             tc_.tile_pool(name="apool", bufs=2) as apool, \
             tc_.tile_pool(name="spool", bufs=8) as spool, \
             tc_.tile_pool(name="opool", bufs=3) as opool, \
             tc_.tile_pool(name="ps1", bufs=2, space="PSUM") as ps1, \
             tc_.tile_pool(name="ps2", bufs=4, space="PSUM") as ps2:

            gupg_r = wpool.tile([128, NJ * KC * 128], f16)
            gupu_r = wpool.tile([128, NJ * KC * 128], f16)
            dwT_r = wpool.tile([128, KC * H], f16)
            gb_r = wpool.tile([128, NJ], f32)
            ub_r = wpool.tile([128, NJ], f32)
            w_r = wpool.tile([128, T // 128], f32)

            # DMA order matches consumption order: tiny bias/route tensors,
            # then interleaved (hs0 kc-slice, gate j-block) pairs so the gate
            # pass can start after the first ~1.3 MB, then the up half, the
            # down weights, and the chunk-1 token prefetch.
            nc.sync.dma_start(gb_r[:], gb[:])
            nc.sync.dma_start(ub_r[:], ub[:])
            nc.sync.dma_start(w_r[:], wt[:])
            hs0 = hpool.tile([128, KC * TC], f16, tag="hs")
            JB = KC * 128   # columns per (j) block of gup
            for k in range(KC):
                nc.sync.dma_start(hs0[:, k*TC:(k+1)*TC],
                                  hsT[k*128:(k+1)*128, 0:TC])
                nc.sync.dma_start(gupg_r[:, k*JB:(k+1)*JB], gupg[:, k*JB:(k+1)*JB])
            for j in range(NJ):
                nc.sync.dma_start(gupu_r[:, j*JB:(j+1)*JB], gupu[:, j*JB:(j+1)*JB])
            nc.sync.dma_start(dwT_r[:], dwT[:])
            hs1 = hpool.tile([128, KC * TC], f16, tag="hs")
            nc.sync.dma_start(
                hs1[:].rearrange("p (kc t) -> p kc t", t=TC),
                hsT[:, TC:2*TC].rearrange("(kc p) t -> p kc t", p=128))

            for c in range(NCHUNK):
                t_off = c * TC
                if c == 0:
                    hs_r = hs0
                elif c == 1:
                    hs_r = hs1
                else:
                    hs_r = hpool.tile([128, KC * TC], f16, tag="hs")
                    nc.sync.dma_start(
                        hs_r[:].rearrange("p (kc t) -> p kc t", t=TC),
                        hsT[:, t_off:t_off + TC].rearrange("(kc p) t -> p kc t", p=128))

                act_r = apool.tile([128, NJ * TC], f16, tag="act")
                s2s = []
                for j in range(NJ):     # gate pass
                    pg = ps1.tile([128, TC], f32, tag="pg")
                    for kc in range(KC):
                        nc.tensor.matmul(pg[:], gupg_r[:, (j*KC + kc)*128:(j*KC + kc + 1)*128],
                                         hs_r[:, kc*TC:(kc+1)*TC],
                                         start=(kc == 0), stop=(kc == KC - 1))
                    s2 = spool.tile([128, TC], f32, tag="s2")
                    nc.scalar.activation(s2[:], pg[:], AF.Silu,
                                         bias=gb_r[:, j:j+1], scale=1.702)
                    s2s.append(s2)
                for j in range(NJ):     # up pass: act = (up + ub + 1) * silu_out
                    pu = ps1.tile([128, TC], f32, tag="pu")
                    for kc in range(KC):
                        nc.tensor.matmul(pu[:], gupu_r[:, (j*KC + kc)*128:(j*KC + kc + 1)*128],
                                         hs_r[:, kc*TC:(kc+1)*TC],
                                         start=(kc == 0), stop=(kc == KC - 1))
                    nc.vector.scalar_tensor_tensor(act_r[:, j*TC:(j+1)*TC], pu[:],
                                                   ub_r[:, j:j+1], s2s[j][:],
                                                   op0=ALU.add, op1=ALU.mult)

                for tt in range(TC // 128):
                    gt = (t_off // 128) + tt
                    wcol = w_r[:, gt:gt+1]
                    ot = opool.tile([128, H], f32, tag="ot")
                    for hh in range(H // 512):
                        p2 = ps2.tile([128, 512], f32, tag="p2")
                        for ic in range(KC):
                            nc.tensor.matmul(p2[:], act_r[:, ic*TC + tt*128:ic*TC + (tt+1)*128],
                                             dwT_r[:, ic*H + hh*512:ic*H + (hh+1)*512],
                                             start=(ic == 0), stop=(ic == KC - 1))
                        nc.vector.tensor_scalar_mul(ot[:, hh*512:(hh+1)*512], p2[:], wcol)
                    nc.sync.dma_start(out[t_off + tt*128:t_off + (tt+1)*128, :], ot[:])
    nc.compile()
    return nc


def _get_nc():
    if 'nc' not in _CACHE:
        _CACHE['nc'] = _build()
    return _CACHE['nc']


def _make_in_maps(hidden_states, routing_weights, gate_up_proj, gate_up_proj_bias,
                  down_proj, down_proj_bias):
    hs = np.asarray(hidden_states, dtype=np.float32)
    rw = np.asarray(routing_weights, dtype=np.float32)
    gupw = np.asarray(gate_up_proj, dtype=np.float32)
    gupb = np.asarray(gate_up_proj_bias, dtype=np.float32)
    dw = np.asarray(down_proj, dtype=np.float32)
    hsT = np.ascontiguousarray(hs.T.astype(np.float16))
    in_maps = []
    for e in range(N_CORES):
        g = gupw[e]
        # (j, kc)-block layout: col block j*KC+kc holds gate[kc*128:(kc+1)*128,
        # j*128:(j+1)*128], so the j-loop consumes weights in DMA order.
        gate = g[:, 0::2].astype(np.float16)
        up = g[:, 1::2].astype(np.float16)
        gate_b = gate.reshape(KC, 128, NJ, 128).transpose(1, 2, 0, 3).reshape(128, NJ*KC*128)
        up_b = up.reshape(KC, 128, NJ, 128).transpose(1, 2, 0, 3).reshape(128, NJ*KC*128)
        # silu(1.702*(x + b)) = silu(1.702*x + 1.702*b); the 1/1.702 glu
        # scale is folded into dwT below.
        dwTe = (dw[e].T / np.float32(1.702)).astype(np.float16)
        in_maps.append({
            "hsT": hsT,
            "gupg": np.ascontiguousarray(gate_b),
            "gupu": np.ascontiguousarray(up_b),
            "gb": np.ascontiguousarray((1.702 * gupb[e, 0::2]).reshape(NJ, 128).T),
            "ub": np.ascontiguousarray((gupb[e, 1::2] + 1.0).reshape(NJ, 128).T),
            "dwT": np.ascontiguousarray(dwTe.reshape(KC, 128, H).transpose(1, 0, 2).reshape(128, KC*H)),
            "wt": np.ascontiguousarray(rw[:, e].reshape(T // 128, 128).T),
        })
    return in_maps


def _assemble(results, routing_weights, down_proj_bias):
    # Sum the 8 expert partials and add the routing-weighted down-bias term
    # (both in fp32 on the host).
    rw = np.asarray(routing_weights, dtype=np.float32)
    db = np.asarray(down_proj_bias, dtype=np.float32)
    out = rw @ db  # [T, E] @ [E, H]
    for r in range(N_CORES):
        out += results[r]["out"]
    return out


def kernel(hidden_states, routing_weights, gate_up_proj, gate_up_proj_bias,
           down_proj, down_proj_bias):
    from concourse import bass_utils
    in_maps = _make_in_maps(hidden_states, routing_weights, gate_up_proj,
                            gate_up_proj_bias, down_proj, down_proj_bias)
    nc = _get_nc()
    try:
        res = bass_utils.run_bass_kernel_spmd(nc, in_maps, core_ids=list(range(N_CORES)))
    except Exception:
        # One retry in case a previous process left a core wedged.
        res = bass_utils.run_bass_kernel_spmd(nc, in_maps, core_ids=list(range(N_CORES)))
    return _assemble(res.results, routing_weights, down_proj_bias)


# revision 9
# speedup vs baseline: 1.4041x; 1.0319x over previous
"""MoE dense all-experts (GPT-OSS Experts forward) on 8 Trainium2 NeuronCores.

Expert-parallel sharding: core e holds expert e's weights and computes its
weighted contribution

    partial_e[t, h] = w[t, e] * ((up + 1) * silu(1.702 * gate) @ down_e.T)

with [gate | up] = hs @ gup_e + bias (the host de-interleaves gup's even/odd
columns so gate/up become contiguous halves). Each core writes its full
[T, H] partial to DRAM; the host sums the 8 partials and adds the
routing-weighted down-bias term (routing_weights @ down_bias) in fp32.

All matmul operands are float16 (fp32 PSUM accumulation): fp32/fp32r moving
operands stream through the PE at ~0.56 ns/col while 16-bit operands stream
at 1 col/cycle @ 2.4 GHz (0.417 ns/col), a 1.35x speedup at identical MAC
count; fp16's 10-bit mantissa keeps the end-to-end relative error ~5e-4.
Weights are staged in (j, kc)-block order so the gate pass starts after
~1.3 MB of DMA instead of waiting for the full weight tensor.

Stage 1 computes [f, t] tiles (gate pass feeding the ScalarE Silu LUT, then
up pass fused with the silu output via scalar_tensor_tensor into fp16
act[i, t]); stage 2 computes out[t, h] = (act.T @ dwT) * w[t] with act as
the stationary operand and a single VectorE tensor_scalar epilogue.
"""
import sys
if '/opt/trn_rl_repo' not in sys.path:
    sys.path.insert(0, '/opt/trn_rl_repo')
import numpy as np

E, H, I, T = 8, 1024, 1024, 4096
N_CORES = 8
TC = 512               # token chunk
NCHUNK = T // TC
KC = H // 128          # contraction chunks (H == I == 1024)
NJ = I // 128          # gate/up row tiles

_CACHE = {}


def _build():
    import concourse.bacc as bacc
    import concourse.tile as tile
    import concourse.mybir as mybir
    f32 = mybir.dt.float32
    f16 = mybir.dt.float16
    AF = mybir.ActivationFunctionType
    ALU = mybir.AluOpType

    nc = bacc.Bacc("TRN2", target_bir_lowering=False, debug=False,
                   enable_asserts=False, num_devices=N_CORES)
    hsT = nc.dram_tensor("hsT", [H, T], f16, kind="ExternalInput").ap()
    gupg = nc.dram_tensor("gupg", [128, NJ * KC * 128], f16, kind="ExternalInput").ap()
    gupu = nc.dram_tensor("gupu", [128, NJ * KC * 128], f16, kind="ExternalInput").ap()
    # misc packs [gb | ub | wt] to save DMA-issue slots (~0.6us each)
    misc = nc.dram_tensor("misc", [128, 2 * NJ + T // 128], f32, kind="ExternalInput").ap()
    dwT = nc.dram_tensor("dwT", [128, KC * H], f16, kind="ExternalInput").ap()
    out = nc.dram_tensor("out", [T, H], f32, kind="ExternalOutput").ap()

    with tile.TileContext(nc) as tc_:
        with tc_.tile_pool(name="wpool", bufs=1) as wpool, \
             tc_.tile_pool(name="hpool", bufs=2) as hpool, \
             tc_.tile_pool(name="apool", bufs=2) as apool, \
             tc_.tile_pool(name="spool", bufs=8) as spool, \
             tc_.tile_pool(name="opool", bufs=3) as opool, \
             tc_.tile_pool(name="ps0", bufs=1, space="PSUM") as ps0, \
             tc_.tile_pool(name="ps1", bufs=2, space="PSUM") as ps1, \
             tc_.tile_pool(name="ps2", bufs=3, space="PSUM") as ps2:

            gupg_r = wpool.tile([128, NJ * KC * 128], f16)
            gupu_r = wpool.tile([128, NJ * KC * 128], f16)
            dwT_r = wpool.tile([128, KC * H], f16)
            misc_r = wpool.tile([128, 2 * NJ + T // 128], f32)
            gb_r = misc_r[:, 0:NJ]
            ub_r = misc_r[:, NJ:2*NJ]
            w_r = misc_r[:, 2*NJ:]
            dummy = wpool.tile([128, 512], f16)
            pdum = ps0.tile([128, 512], f32)

            # DMA order matches consumption order: the chunk-0 tokens, the
            # bias/route pack, then gate j-blocks (the gate pass consumes them
            # in this order), the up half, down weights, chunk-1 prefetch.
            hs0 = hpool.tile([128, KC * TC], f16, tag="hs")
            JB = KC * 128   # columns per (j) block of gup
            nc.sync.dma_start(
                hs0[:].rearrange("p (kc t) -> p kc t", t=TC),
                hsT[:, 0:TC].rearrange("(kc p) t -> p kc t", p=128))
            nc.sync.dma_start(misc_r[:], misc[:])
            for j in range(NJ):
                nc.sync.dma_start(gupg_r[:, j*JB:(j+1)*JB], gupg[:, j*JB:(j+1)*JB])
            for m in range(4):
                nc.sync.dma_start(gupu_r[:, 2*m*JB:2*(m+1)*JB], gupu[:, 2*m*JB:2*(m+1)*JB])
            nc.sync.dma_start(dwT_r[:, :KC*H//2], dwT[:, :KC*H//2])
            nc.sync.dma_start(dwT_r[:, KC*H//2:], dwT[:, KC*H//2:])
            hs1 = hpool.tile([128, KC * TC], f16, tag="hs")
            nc.sync.dma_start(
                hs1[:].rearrange("p (kc t) -> p kc t", t=TC),
                hsT[:, TC:2*TC].rearrange("(kc p) t -> p kc t", p=128))

            # PE warm-up: ~16 dependency-free matmuls fill the HAM activity
            # window (3.4us) during the DMA preamble so the real matmuls start
            # at 2.4 GHz instead of 1.2 GHz, and the PE never idles long
            # enough mid-preamble to re-throttle.
            nc.vector.memset(dummy[:], 0)
            for i in range(16):
                nc.tensor.matmul(pdum[:], dummy[:, 0:128], dummy[:],
                                 start=(i == 0), stop=(i == 15))

            for c in range(NCHUNK):
                t_off = c * TC
                if c == 0:
                    hs_r = hs0
                elif c == 1:
                    hs_r = hs1
                else:
                    hs_r = hpool.tile([128, KC * TC], f16, tag="hs")
                    nc.sync.dma_start(
                        hs_r[:].rearrange("p (kc t) -> p kc t", t=TC),
                        hsT[:, t_off:t_off + TC].rearrange("(kc p) t -> p kc t", p=128))

                act_r = apool.tile([128, NJ * TC], f16, tag="act")
                s2s = []
                for j in range(NJ):     # gate pass
                    pg = ps1.tile([128, TC], f32, tag="pg")
                    for kc in range(KC):
                        nc.tensor.matmul(pg[:], gupg_r[:, (j*KC + kc)*128:(j*KC + kc + 1)*128],
                                         hs_r[:, kc*TC:(kc+1)*TC],
                                         start=(kc == 0), stop=(kc == KC - 1))
                    s2 = spool.tile([128, TC], f32, tag="s2")
                    nc.scalar.activation(s2[:], pg[:], AF.Silu,
                                         bias=gb_r[:, j:j+1], scale=1.702)
                    s2s.append(s2)
                for j in range(NJ):     # up pass: act = (up + ub + 1) * silu_out
                    pu = ps1.tile([128, TC], f32, tag="pu")
                    for kc in range(KC):
                        nc.tensor.matmul(pu[:], gupu_r[:, (j*KC + kc)*128:(j*KC + kc + 1)*128],
                                         hs_r[:, kc*TC:(kc+1)*TC],
                                         start=(kc == 0), stop=(kc == KC - 1))
                    nc.vector.scalar_tensor_tensor(act_r[:, j*TC:(j+1)*TC], pu[:],
                                                   ub_r[:, j:j+1], s2s[j][:],
                                                   op0=ALU.add, op1=ALU.mult)

                for tt in range(TC // 128):
                    gt = (t_off // 128) + tt
                    wcol = w_r[:, gt:gt+1]
                    ot = opool.tile([128, H], f32, tag="ot")
                    for hh in range(H // 512):
                        p2 = ps2.tile([128, 512], f32, tag="p2")
                        for ic in range(KC):
                            nc.tensor.matmul(p2[:], act_r[:, ic*TC + tt*128:ic*TC + (tt+1)*128],
                                             dwT_r[:, ic*H + hh*512:ic*H + (hh+1)*512],
                                             start=(ic == 0), stop=(ic == KC - 1))
                        nc.vector.tensor_scalar_mul(ot[:, hh*512:(hh+1)*512], p2[:], wcol)
                        # per-half DMA so the final transfer trails the last
                        # matmul by ~1.5us instead of ~4us
                        nc.sync.dma_start(
                            out[t_off + tt*128:t_off + (tt+1)*128, hh*512:(hh+1)*512],
                            ot[:, hh*512:(hh+1)*512])
    nc.compile()
    return nc


def _get_nc():
    if 'nc' not in _CACHE:
        _CACHE['nc'] = _build()
    return _CACHE['nc']


def _make_in_maps(hidden_states, routing_weights, gate_up_proj, gate_up_proj_bias,
                  down_proj, down_proj_bias):
    hs = np.asarray(hidden_states, dtype=np.float32)
    rw = np.asarray(routing_weights, dtype=np.float32)
    gupw = np.asarray(gate_up_proj, dtype=np.float32)
    gupb = np.asarray(gate_up_proj_bias, dtype=np.float32)
    dw = np.asarray(down_proj, dtype=np.float32)
    hsT = np.ascontiguousarray(hs.T.astype(np.float16))
    in_maps = []
    for e in range(N_CORES):
        g = gupw[e]
        # (j, kc)-block layout: col block j*KC+kc holds gate[kc*128:(kc+1)*128,
        # j*128:(j+1)*128], so the j-loop consumes weights in DMA order.
        gate = g[:, 0::2].astype(np.float16)
        up = g[:, 1::2].astype(np.float16)
        gate_b = gate.reshape(KC, 128, NJ, 128).transpose(1, 2, 0, 3).reshape(128, NJ*KC*128)
        up_b = up.reshape(KC, 128, NJ, 128).transpose(1, 2, 0, 3).reshape(128, NJ*KC*128)
        # silu(1.702*(x + b)) = silu(1.702*x + 1.702*b); the 1/1.702 glu
        # scale is folded into dwT below.
        dwTe = (dw[e].T / np.float32(1.702)).astype(np.float16)
        misc = np.concatenate([
            (1.702 * gupb[e, 0::2]).reshape(NJ, 128).T,
            (gupb[e, 1::2] + 1.0).reshape(NJ, 128).T,
            rw[:, e].reshape(T // 128, 128).T,
        ], axis=1).astype(np.float32)
        in_maps.append({
            "hsT": hsT,
            "gupg": np.ascontiguousarray(gate_b),
            "gupu": np.ascontiguousarray(up_b),
            "misc": np.ascontiguousarray(misc),
            "dwT": np.ascontiguousarray(dwTe.reshape(KC, 128, H).transpose(1, 0, 2).reshape(128, KC*H)),
        })
    return in_maps


def _assemble(results, routing_weights, down_proj_bias):
    # Sum the 8 expert partials and add the routing-weighted down-bias term
    # (both in fp32 on the host).
    rw = np.asarray(routing_weights, dtype=np.float32)
    db = np.asarray(down_proj_bias, dtype=np.float32)
    out = rw @ db  # [T, E] @ [E, H]
    for r in range(N_CORES):
        out += results[r]["out"]
    return out


def kernel(hidden_states, routing_weights, gate_up_proj, gate_up_proj_bias,
           down_proj, down_proj_bias):
    from concourse import bass_utils
    in_maps = _make_in_maps(hidden_states, routing_weights, gate_up_proj,
                            gate_up_proj_bias, down_proj, down_proj_bias)
    nc = _get_nc()
    try:
        res = bass_utils.run_bass_kernel_spmd(nc, in_maps, core_ids=list(range(N_CORES)))
    except Exception:
        # One retry in case a previous process left a core wedged.
        res = bass_utils.run_bass_kernel_spmd(nc, in_maps, core_ids=list(range(N_CORES)))
    return _assemble(res.results, routing_weights, down_proj_bias)
